# revision 19
# baseline (speedup 1.0000x reference)
"""3-layer GCN (PyG GCNConv x3 + softmax/log_softmax) on 8 Trainium2 NeuronCores.

Strategy (graph/data parallel, sharded by dst node range):
  - Fold the symmetric norm into node features: g = dinv * h. Then
      gcn_conv(h)[i] = dinv[i] * (sum_{e: dst=i} g[src_e] + g[i]) + b
  - Each core owns N/8 nodes. Per layer:
      dense:   d = h_own @ W (TensorE), g_own = dinv*d (ScalarE)
      AG:      AllGather g_own -> full G table in DRAM (bf16)
      scatter: edges sorted by dst tile; per 128-edge block: dma_gather rows
               G[src] -> SBUF, matmul with a 0/1 one-hot lhsT (S block) into
               PSUM; self loop = identity-matmul of g_own tile (first, start=True)
      epilogue: h = relu(dinv * psum) via one ScalarE activation
  - h_own is transposed on TensorE between layers (dense needs feat-major lhsT).
  - Final layer: z = dinv*psum, then softmax + log_softmax rowwise on chip.

Precision: bf16 operands, fp32 PSUM accumulation, fp32 epilogue/softmax.
"""

import os
import sys
from dataclasses import dataclass, field

import numpy as np
import ml_dtypes

import concourse.bass as bass
import concourse.bacc as bacc
import concourse.tile as tile
import concourse.mybir as mybir
from concourse import library_config
from concourse.bass_utils import run_bass_kernel_spmd

BF16 = mybir.dt.bfloat16
F32 = mybir.dt.float32
I16 = mybir.dt.int16
NPBF16 = ml_dtypes.bfloat16


def _ceil_div(a, b):
    return (a + b - 1) // b


def _round_up(a, b):
    return _ceil_div(a, b) * b


@dataclass
class Cfg:
    n_nodes: int = 50000
    n_cores: int = 8
    d_in: int = 512       # multiple of 128
    d_hid: int = 256      # multiple of 128
    d_out: int = 64       # <= 128
    d_out_pad: int = 128  # gather elem must be >=256B -> pad L3 feature dim

    @property
    def v(self):  # nodes per core
        assert self.n_nodes % self.n_cores == 0
        return self.n_nodes // self.n_cores

    @property
    def vp(self):  # padded nodes per core
        return _round_up(self.v, 128)

    @property
    def t(self):  # dst tiles per core
        return self.vp // 128

    @property
    def rows(self):  # G table rows
        return self.n_cores * self.vp

    @property
    def half_rows(self):  # table split point (int16 gather index limit)
        return (self.n_cores // 2) * self.vp


@dataclass
class Struct:
    """Static (compile-time) per-tile block structure, shared by all cores."""
    ka: list = field(default_factory=list)  # blocks of table-A edges per tile
    kb: list = field(default_factory=list)  # blocks of table-B edges per tile
    any_bias: bool = False

    @property
    def totblk(self):
        return sum(self.ka) + sum(self.kb)

    @property
    def totslot(self):
        return self.totblk * 128

    def tile_block_off(self, ti):
        return sum(self.ka[:ti]) + sum(self.kb[:ti])


def preprocess(cfg: Cfg, x, edge_index, W1, b1, W2, b2, W3, b3):
    """Host-side: shard + build all per-core device input arrays."""
    C, V, VP, T = cfg.n_cores, cfg.v, cfg.vp, cfg.t
    N = cfg.n_nodes

    src = np.asarray(edge_index[0], dtype=np.int64)
    dst = np.asarray(edge_index[1], dtype=np.int64)
    E = src.shape[0]

    deg = 1.0 + np.bincount(dst, minlength=N).astype(np.float64)
    dinv = (1.0 / np.sqrt(deg)).astype(np.float32)

    # G-table row of a source node s (partition-major per-rank layout):
    # rank r = s // V, local l = s % V, tile t = l // 128, lane p = l % 128
    # row = r*VP + p*T + t   (matches the [128, T, F] per-rank DMA layout)
    s_r = src // V
    s_l = src % V
    srow = s_r * VP + (s_l % 128) * T + (s_l // 128)
    in_b = srow >= cfg.half_rows  # which table half

    d_c = dst // V
    d_l = dst % V
    d_t = d_l // 128
    d_loc = d_l % 128

    # group edges by (core, tile, half)
    key = (d_c * T + d_t) * 2 + in_b.astype(np.int64)
    cnt = np.bincount(key, minlength=C * T * 2).reshape(C, T, 2)
    ka_per = _ceil_div(cnt[:, :, 0], 128)  # [C, T]
    kb_per = _ceil_div(cnt[:, :, 1], 128)
    st = Struct(
        ka=[int(k) for k in ka_per.max(axis=0)],
        kb=[int(k) for k in kb_per.max(axis=0)],
        any_bias=bool(np.any(b1) or np.any(b2) or np.any(b3)),
    )

    order = np.argsort(key, kind="stable")
    # rank of each edge within its (c,t,half) group
    sorted_key = key[order]
    group_start = np.zeros(C * T * 2, dtype=np.int64)
    np.cumsum(np.bincount(sorted_key, minlength=C * T * 2)[:-1], out=group_start[1:])
    rank_in_group = np.arange(E, dtype=np.int64) - group_start[sorted_key]

    # slot offsets
    blkoff_a = np.zeros((T,), dtype=np.int64)  # block offset of tile's A blocks
    blkoff_b = np.zeros((T,), dtype=np.int64)
    acc = 0
    for ti in range(T):
        blkoff_a[ti] = acc
        blkoff_b[ti] = acc + st.ka[ti]
        acc += st.ka[ti] + st.kb[ti]
    assert acc == st.totblk

    TOTBLK, TOTSLOT = st.totblk, st.totslot

    e_sorted = order
    core_s = d_c[e_sorted]
    tile_s = d_t[e_sorted]
    half_s = in_b[e_sorted]
    slotbase = np.where(half_s, blkoff_b[tile_s], blkoff_a[tile_s]) * 128
    slot_s = slotbase + rank_in_group
    srow_rel = np.where(half_s, srow[e_sorted] - cfg.half_rows, srow[e_sorted])
    dloc_s = d_loc[e_sorted]

    per_core = []
    for c in range(C):
        m = core_s == c
        slots = slot_s[m]
        # gather indices (int16), padding slots point at row 0 of their half
        gidx = np.zeros((TOTSLOT,), dtype=np.int16)
        gidx[slots] = srow_rel[m].astype(np.int16)
        # wrap in 16 partitions, replicate to 128
        g16 = gidx.reshape(TOTSLOT // 16, 16).T  # [16, TOTSLOT//16]
        gidx128 = np.tile(g16, (8, 1)).copy()  # [128, TOTSLOT//16]

        # one-hot S blocks, partition-major: s128[p, blk, j] = 1 if slot
        # (blk*128+p) has dstloc j
        s128 = np.zeros((128, TOTBLK, 128), dtype=NPBF16)
        s128[slots % 128, slots // 128, dloc_s[m]] = NPBF16(1.0)

        # dinvT [128, T]
        dv = np.zeros((128, T), dtype=np.float32)
        lidx = np.arange(V)
        dv[lidx % 128, lidx // 128] = dinv[c * V:(c + 1) * V]

        # xT slabs [128, d_in//128, VP]
        ks1 = cfg.d_in // 128
        xp = np.zeros((VP, cfg.d_in), dtype=np.float32)
        xp[:V] = np.asarray(x[c * V:(c + 1) * V], dtype=np.float32)
        xt = np.ascontiguousarray(
            xp.T.reshape(ks1, 128, VP).transpose(1, 0, 2)
        ).astype(NPBF16)

        per_core.append({
            "xT": xt.reshape(128, -1),
            "gidx": gidx128,
            "sblk": s128.reshape(128, -1),
            "dinvT": dv,
        })

    # shared (replicated) weights
    def wfmt(W, dpad=None):
        W = np.asarray(W, dtype=np.float32)
        kin, kout = W.shape
        if dpad is not None and kout < dpad:
            W = np.concatenate([W, np.zeros((kin, dpad - kout), np.float32)], axis=1)
            kout = dpad
        ks = kin // 128
        return np.ascontiguousarray(
            W.reshape(ks, 128, kout).transpose(1, 0, 2)
        ).astype(NPBF16).reshape(128, -1)

    shared = {
        "w1": wfmt(W1),
        "w2": wfmt(W2),
        "w3": wfmt(W3, dpad=cfg.d_out_pad),
        "ident": np.eye(128, dtype=NPBF16),
    }
    if st.any_bias:
        shared["b1"] = np.asarray(b1, np.float32).reshape(1, -1)
        shared["b2"] = np.asarray(b2, np.float32).reshape(1, -1)
        b3p = np.zeros((1, cfg.d_out_pad), np.float32)
        b3p[0, :cfg.d_out] = np.asarray(b3, np.float32)
        shared["b3"] = b3p

    in_maps = []
    for c in range(C):
        m = dict(per_core[c])
        m.update(shared)
        in_maps.append(m)
    return st, in_maps


def build_program(cfg: Cfg, st: Struct, ag_mode: str = "cc", stages: str = "full",
                  gchunk: int = 0, n_devices_override: int | None = None):
    """Build the Bass/Tile program (same NEFF for all cores).

    ag_mode: "cc" = real AllGather collective; "local" = debug mode that
    replicates the local slice into every table slot (wrong results for
    cross-core edges, used only to bisect hangs).
    """
    C, VP, T = cfg.n_cores, cfg.vp, cfg.t
    ROWS, HALF = cfg.rows, cfg.half_rows
    DH, DOP = cfg.d_hid, cfg.d_out_pad
    KS1, KS2 = cfg.d_in // 128, cfg.d_hid // 128
    TOTBLK, TOTSLOT = st.totblk, st.totslot

    nc = bacc.Bacc("TRN2", target_bir_lowering=False, debug=False,
                   num_devices=n_devices_override or C)

    # ---- I/O ----
    xT_d = nc.dram_tensor("xT", [128, KS1 * VP], BF16, kind="ExternalInput").ap()
    w_d = [
        nc.dram_tensor("w1", [128, KS1 * DH], BF16, kind="ExternalInput").ap(),
        nc.dram_tensor("w2", [128, KS2 * DH], BF16, kind="ExternalInput").ap(),
        nc.dram_tensor("w3", [128, KS2 * DOP], BF16, kind="ExternalInput").ap(),
    ]
    dinvT_d = nc.dram_tensor("dinvT", [128, T], F32, kind="ExternalInput").ap()
    sblk_d = nc.dram_tensor("sblk", [128, TOTBLK * 128], BF16, kind="ExternalInput").ap()
    gidx_d = nc.dram_tensor("gidx", [128, TOTSLOT // 16], I16, kind="ExternalInput").ap()
    ident_d = nc.dram_tensor("ident", [128, 128], BF16, kind="ExternalInput").ap()
    b_d = None
    if st.any_bias:
        b_d = [
            nc.dram_tensor("b1", [1, DH], F32, kind="ExternalInput").ap(),
            nc.dram_tensor("b2", [1, DH], F32, kind="ExternalInput").ap(),
            nc.dram_tensor("b3", [1, DOP], F32, kind="ExternalInput").ap(),
        ]
    out_d = nc.dram_tensor("out", [VP, cfg.d_out], F32, kind="ExternalOutput").ap()

    F_of = [DH, DH, DOP]       # layer output feature dims (padded)
    KS_of = [KS1, KS2, KS2]    # contraction slabs per layer

    with tile.TileContext(nc) as tc:
        with (
            tc.tile_pool(name="const", bufs=1) as constp,
            tc.tile_pool(name="hT", bufs=1) as hTp,
            tc.tile_pool(name="gown", bufs=1) as gownp,
            tc.tile_pool(name="sblkp", bufs=3) as sp,
            tc.tile_pool(name="gath", bufs=3) as gp,
            tc.tile_pool(name="htile", bufs=3) as hp,
            tc.tile_pool(name="eptmp", bufs=4) as ep,
            tc.tile_pool(name="psum_mm", bufs=4, space="PSUM") as pmm,
            tc.tile_pool(name="psum_tr", bufs=4, space="PSUM") as ptr,
            tc.tile_pool(name="dram", bufs=1, space="DRAM") as dramp,
        ):
            # ---- constants ----

            # one shared register per distinct gather length (avoid register
            # exhaustion: dma_gather's to_reg allocates otherwise)
            _nreg = {}

            def nidx_reg(n):
                if n not in _nreg:
                    _nreg[n] = nc.gpsimd.to_reg(n)
                return _nreg[n]

            dinv_sb = constp.tile([128, T], F32)
            nc.sync.dma_start(dinv_sb[:], dinvT_d)
            ident_sb = constp.tile([128, 128], BF16)
            nc.sync.dma_start(ident_sb[:], ident_d)
            gidx_sb = constp.tile([128, TOTSLOT // 16], I16)
            nc.sync.dma_start(gidx_sb[:], gidx_d)
            w_sb = []
            for li in range(3):
                w = constp.tile([128, KS_of[li] * F_of[li]], BF16, name=f"w{li}_sb")
                nc.sync.dma_start(w[:], w_d[li])
                w_sb.append(w)
            bias_sb = None
            if st.any_bias:
                bias_sb = []
                for li in range(3):
                    bt = constp.tile([128, F_of[li]], F32, name=f"b{li}_sb")
                    nc.sync.dma_start(bt[:1, :], b_d[li])
                    nc.gpsimd.partition_broadcast(bt[:], bt[:1, :])
                    bias_sb.append(bt)

            # ---- per-layer DRAM: g slice + gathered table ----
            g_dram = [
                dramp.tile([128, T * DH], BF16, name="g1d"),
                dramp.tile([128, T * DH], BF16, name="g2d"),
                dramp.tile([128, T * DOP], BF16, name="g3d"),
            ]
            g_addr_space = "Shared" if ag_mode == "cc" else "Local"
            G_tab = [
                dramp.tile([ROWS, DH], BF16, name="G1", addr_space=g_addr_space),
                dramp.tile([ROWS, DH], BF16, name="G2", addr_space=g_addr_space),
                dramp.tile([ROWS, DOP], BF16, name="G3", addr_space=g_addr_space),
            ]

            # hT tile: stationary operand for dense (feat-major current h)
            hT = hTp.tile([128, KS1 * VP], BF16, tag="hT")
            nc.sync.dma_start(hT[:], xT_d)

            for li in range(3):
                F = F_of[li]
                KS = KS_of[li]
                hT3 = hT.rearrange("p (k n) -> p k n", k=KS)
                w3 = w_sb[li].rearrange("p (k f) -> p k f", k=KS)

                g_own = gownp.tile([128, T * F], BF16, tag="g_own", name=f"g_own{li}")
                g_own3 = g_own.rearrange("p (t f) -> p t f", t=T)
                gd3 = g_dram[li].rearrange("p (t f) -> p t f", t=T)

                # ---------- dense: g = dinv * (h @ W) ----------
                for ti in range(T):
                    psd = pmm.tile([128, F], F32, tag="psum", name="psd")
                    for k in range(KS):
                        nc.tensor.matmul(
                            psd[:],
                            lhsT=hT3[:, k, ti * 128:(ti + 1) * 128],
                            rhs=w3[:, k, :],
                            start=(k == 0),
                            stop=(k == KS - 1),
                        )
                    nc.scalar.activation(
                        g_own3[:, ti, :], psd[:],
                        mybir.ActivationFunctionType.Copy,
                        scale=dinv_sb[:, ti:ti + 1],
                    )
                    nc.sync.dma_start(gd3[:, ti, :], g_own3[:, ti, :])

                # ---------- AllGather g -> G table ----------
                if ag_mode == "cc":
                    nc.gpsimd.collective_compute(
                        "AllGather",
                        mybir.AluOpType.bypass,
                        replica_groups=[list(range(C))],
                        ins=[g_dram[li][:].opt()],
                        outs=[G_tab[li][:].opt()],
                    )
                else:  # debug: fill table with local copies (wrong data)
                    Gr = G_tab[li].rearrange("(c r) f -> c r f", c=C)
                    gl = g_dram[li].rearrange("p (t f) -> (p t) f", t=T)
                    for c in range(C):
                        nc.sync.dma_start(Gr[c], gl)

                tabA = G_tab[li][0:HALF, :]
                tabB = G_tab[li][HALF:ROWS, :]

                if li < 2:
                    hT_next = hTp.tile(
                        [128, KS2 * VP], BF16, tag="hT", name=f"hT{li + 1}"
                    )
                    hTn3 = hT_next.rearrange("p (k n) -> p k n", k=KS2)

                # ---------- scatter + epilogue per dst tile ----------
                for ti in range(T):
                    ka, kb = st.ka[ti], st.kb[ti]
                    nb = ka + kb
                    boff = st.tile_block_off(ti)
                    soff16 = boff * 8  # slot offset / 16

                    ps = pmm.tile([128, F], F32, tag="psum", name="ps")
                    # self loop: psum = I.T @ g_own[tile]  (also zeroes psum)
                    nc.tensor.matmul(
                        ps[:], lhsT=ident_sb[:], rhs=g_own3[:, ti, :],
                        start=True, stop=(nb == 0 or stages == "noscatter"),
                    )

                    if nb > 0 and stages != "noscatter":
                        s_sb = sp.tile([128, nb * 128], BF16, tag="s_sb")
                        nc.sync.dma_start(
                            s_sb[:],
                            sblk_d[:, boff * 128:(boff + nb) * 128],
                        )
                        s_sb3 = s_sb.rearrange("p (b j) -> p b j", b=nb)

                        gt = gp.tile([128, nb * F], BF16, tag="gt")
                        gt3 = gt.rearrange("p (b f) -> p b f", b=nb)
                        if stages != "nogather":
                            def emit_gathers(tab, b0, nblk):
                                step = nblk if gchunk <= 0 else gchunk
                                for cb in range(0, nblk, step):
                                    n = min(step, nblk - cb)
                                    o16 = soff16 + (b0 + cb) * 8
                                    nc.gpsimd.dma_gather(
                                        gt3[:, b0 + cb:b0 + cb + n, :], tab,
                                        gidx_sb[:, o16:o16 + n * 8],
                                        num_idxs=n * 128,
                                        num_idxs_reg=nidx_reg(n * 128),
                                        elem_size=F,
                                    )
                            if ka > 0:
                                emit_gathers(tabA, 0, ka)
                            if kb > 0:
                                emit_gathers(tabB, ka, kb)
                        else:
                            nc.vector.memset(gt[:], 0.5)
                        for b in range(nb):
                            nc.tensor.matmul(
                                ps[:],
                                lhsT=s_sb3[:, b, :],
                                rhs=gt3[:, b, :],
                                start=False,
                                stop=(b == nb - 1),
                            )

                    # ---------- epilogue ----------
                    if li < 2:
                        if st.any_bias:
                            tmp = ep.tile([128, F], F32, tag="btmp")
                            nc.vector.tensor_scalar(
                                tmp[:], ps[:], dinv_sb[:, ti:ti + 1], None,
                                op0=mybir.AluOpType.mult,
                            )
                            nc.vector.tensor_tensor(
                                tmp[:], tmp[:], bias_sb[li][:],
                                op=mybir.AluOpType.add,
                            )
                            ht = hp.tile([128, F], BF16, tag="ht")
                            nc.scalar.activation(
                                ht[:], tmp[:], mybir.ActivationFunctionType.Relu,
                            )
                        else:
                            ht = hp.tile([128, F], BF16, tag="ht")
                            nc.scalar.activation(
                                ht[:], ps[:], mybir.ActivationFunctionType.Relu,
                                scale=dinv_sb[:, ti:ti + 1],
                            )
                        # transpose h tile -> feat-major for next dense
                        for kk in range(KS2):
                            pt = ptr.tile([128, 128], BF16, tag="pt")
                            nc.tensor.transpose(
                                pt[:], ht[:, kk * 128:(kk + 1) * 128], ident_sb[:]
                            )
                            nc.vector.tensor_copy(
                                hTn3[:, kk, ti * 128:(ti + 1) * 128], pt[:]
                            )
                    else:
                        DO = cfg.d_out
                        z = ep.tile([128, DO], F32, tag="z")
                        if st.any_bias:
                            tmpz = ep.tile([128, DO], F32, tag="btmpz")
                            nc.vector.tensor_scalar(
                                tmpz[:], ps[:, 0:DO], dinv_sb[:, ti:ti + 1], None,
                                op0=mybir.AluOpType.mult,
                            )
                            nc.vector.tensor_tensor(
                                z[:], tmpz[:], bias_sb[li][:, 0:DO],
                                op=mybir.AluOpType.add,
                            )
                        else:
                            nc.scalar.activation(
                                z[:], ps[:, 0:DO],
                                mybir.ActivationFunctionType.Copy,
                                scale=dinv_sb[:, ti:ti + 1],
                            )
                        # softmax then log_softmax (rowwise)
                        nm = ep.tile([128, 1], F32, tag="nm")
                        nc.vector.tensor_reduce(
                            nm[:], z[:], axis=mybir.AxisListType.X,
                            op=mybir.AluOpType.max, negate=True,
                        )
                        e1 = ep.tile([128, DO], F32, tag="e1")
                        s1 = ep.tile([128, 1], F32, tag="s1")
                        nc.scalar.activation(
                            e1[:], z[:], mybir.ActivationFunctionType.Exp,
                            bias=nm[:, 0:1], accum_out=s1[:, 0:1],
                        )
                        r1 = ep.tile([128, 1], F32, tag="r1")
                        nc.vector.reciprocal(r1[:], s1[:])
                        p1 = ep.tile([128, DO], F32, tag="p1")
                        nc.vector.tensor_scalar(
                            p1[:], e1[:], r1[:, 0:1], None,
                            op0=mybir.AluOpType.mult,
                        )
                        nm2 = ep.tile([128, 1], F32, tag="nm2")
                        nc.vector.tensor_reduce(
                            nm2[:], p1[:], axis=mybir.AxisListType.X,
                            op=mybir.AluOpType.max, negate=True,
                        )
                        e2 = ep.tile([128, DO], F32, tag="e2")
                        s2 = ep.tile([128, 1], F32, tag="s2")
                        nc.scalar.activation(
                            e2[:], p1[:], mybir.ActivationFunctionType.Exp,
                            bias=nm2[:, 0:1], accum_out=s2[:, 0:1],
                        )
                        l2 = ep.tile([128, 1], F32, tag="l2")
                        nc.scalar.activation(
                            l2[:], s2[:], mybir.ActivationFunctionType.Ln,
                        )
                        sh = ep.tile([128, 1], F32, tag="sh")
                        nc.vector.tensor_tensor(
                            sh[:], nm2[:], l2[:], op=mybir.AluOpType.subtract,
                        )
                        ot = ep.tile([128, DO], F32, tag="ot")
                        nc.vector.tensor_scalar(
                            ot[:], p1[:], sh[:, 0:1], None,
                            op0=mybir.AluOpType.add,
                        )
                        nc.sync.dma_start(out_d[ti * 128:(ti + 1) * 128, :], ot[:])

                if li < 2:
                    hT = hT_next

    nc.compile()
    return nc


_CACHE = {}


def _run(cfg: Cfg, inputs: dict, trace: bool = False):
    import hashlib

    key = hashlib.sha256(
        np.ascontiguousarray(inputs["edge_index"]).tobytes()
    ).hexdigest()
    if key in _CACHE:
        st, nc = _CACHE[key]
        _, in_maps = preprocess(cfg, **inputs)
    else:
        st, in_maps = preprocess(cfg, **inputs)
        nc = build_program(cfg, st, gchunk=3)
        _CACHE[key] = (st, nc)

    res = run_bass_kernel_spmd(
        nc, in_maps, core_ids=list(range(cfg.n_cores)), trace=trace,
        trace_cores=list(range(cfg.n_cores)) if trace else None,
        stitch_traces=trace,
    )
    out = np.concatenate(
        [res.results[c]["out"][:cfg.v] for c in range(cfg.n_cores)], axis=0
    )
    return out.astype(np.float32), res


def kernel(**inputs) -> np.ndarray:
    cfg = Cfg()
    out, _ = _run(cfg, inputs, trace=False)
    return out
